# revision 13
# baseline (speedup 1.0000x reference)
"""Pairwise cosine similarity on 8 Trainium2 NeuronCores.

Computes sim[n, m] = <x_n, y_m> / (||x_n|| * ||y_m||) for
input1 [8192, 128], input2 [8192, 128] -> out [8192, 8192] (fp32 API).

Sharding: input1 rows split 8 ways (1024 rows/core); input2 replicated.
Each core computes one [1024, 8192] output stripe; host concatenates.

Host prep does ALL normalization and layout work: rows are scaled by
1/max(||r||, eps) in fp32, transposed to [d, rows], and cast to bf16.
The device kernel is then a pure stream: plain DMA loads of xT/yT,
bf16 matmuls with fp32 PSUM accumulation, PSUM->SBUF bf16 drains
split DVE/ACT by psum-tile parity, and whole-row-block 2 MB stores
(contiguous DRAM regions; the last block stores in 512 KB pieces to
shorten the serial tail). All DMAs issue from the Sync engine so the
first load starts right as the preamble ends (~7 us) and loads/stores
drain strict-FIFO on one HWDGE ring with no SDMA idle.

Per-core HBM traffic: 2.25 MB loads + 16.78 MB stores ~= 53 us floor
at 358 GB/s. Measured rel err ~4e-3 (budget 2e-2).
"""

import numpy as np
import ml_dtypes

import concourse.bass as bass
import concourse.tile as tile
from concourse import bacc, mybir
from concourse.bass_utils import run_bass_kernel_spmd

N_CORES = 8
D = 128          # feature dim == contraction partitions
P = 128          # SBUF partitions
NT = 512         # matmul free dim (one fp32 PSUM bank)
MMCOLS = 1024    # PSUM tile columns (2 banks; pool of 4 => all 8 banks)

F32 = mybir.dt.float32
BF16 = mybir.dt.bfloat16


def build_nc(rows_per_core: int, corpus_rows: int) -> bass.Bass:
    nc = bacc.Bacc(None)

    xT = nc.dram_tensor("xT", [D, rows_per_core], BF16, kind="ExternalInput")
    yT = nc.dram_tensor("yT", [D, corpus_rows], BF16, kind="ExternalInput")
    out = nc.dram_tensor(
        "out", [rows_per_core, corpus_rows], BF16, kind="ExternalOutput"
    )

    nbx = rows_per_core // P          # x row-blocks (8)
    # y load chunks: a small first chunk starts the matmul/drain pipeline
    # early; the big last chunk keeps per-partition DMA lines at 8 KB.
    ychunks = [2048, 2048, 4096]
    if sum(ychunks) != corpus_rows:
        ychunks = [corpus_rows]

    with tile.TileContext(nc) as tc:
        with (
            tc.tile_pool(name="const", bufs=1) as constp,
            tc.tile_pool(name="xt", bufs=1) as xtp,
            tc.tile_pool(name="yt", bufs=1) as ytp,
            tc.tile_pool(name="obuf", bufs=4) as obufp,
            tc.tile_pool(name="mm", bufs=4, space=bass.MemorySpace.PSUM) as mpsum,
        ):
            wt = constp.tile([P, NT], BF16)
            nc.vector.memset(wt[:], 0.0)

            # All loads issue from Sync: it clears the preamble first
            # (~6.8 us) and its HWDGE ring then serves loads-then-stores
            # strict-FIFO with no idle.
            xt = xtp.tile([P, rows_per_core], BF16)
            nc.sync.dma_start(out=xt[:], in_=xT[:])
            yt = ytp.tile([P, corpus_rows], BF16)
            c0 = 0
            for cw in ychunks:
                nc.sync.dma_start(out=yt[:, c0 : c0 + cw], in_=yT[:, c0 : c0 + cw])
                c0 += cw

            # PE keep-warm: dummy bf16 matmuls bridge the load gap so the
            # HAM clock gate opens (2.4 vs 1.2 GHz) before the real stream.
            wps = mpsum.tile([P, MMCOLS], F32, tag="ps")
            for i in range(8):
                nc.tensor.matmul(
                    wps[:, (i % 2) * NT : (i % 2) * NT + NT],
                    wt[:, :P],
                    wt[:],
                    start=True,
                    stop=True,
                )

            # Band-outer over two 4096-col halves: every store is a 1 MB
            # [128, 4096] piece (the measured store-rate sweet spot,
            # ~420 B/ns) and band A pieces depend only on the first two y
            # chunks, so the store stream starts the moment the y load
            # tail clears the ring and never gaps. Drain supply (a
            # DVE/ACT pair per 2048 cols, ~2.26 us per piece) stays just
            # ahead of the ~2.4 us-per-piece store stream.
            BAND = corpus_rows // 2
            for hb in range(2):
                col0 = hb * BAND
                for b in range(nbx):
                    lhs = xt[:, b * P : (b + 1) * P]
                    ob = obufp.tile([P, BAND], BF16, tag="ob")
                    for hh in range(0, BAND, MMCOLS):
                        ps = mpsum.tile([P, MMCOLS], F32, tag="ps")
                        for j in range(0, MMCOLS, NT):
                            nc.tensor.matmul(
                                ps[:, j : j + NT],
                                lhs,
                                yt[:, col0 + hh + j : col0 + hh + j + NT],
                                start=True,
                                stop=True,
                            )
                        if (hh // MMCOLS) % 2 == 0:
                            nc.vector.tensor_copy(ob[:, hh : hh + MMCOLS], ps[:])
                        else:
                            nc.scalar.copy(ob[:, hh : hh + MMCOLS], ps[:])
                    nc.sync.dma_start(
                        out=out[b * P : (b + 1) * P, col0 : col0 + BAND],
                        in_=ob[:],
                    )

    nc.finalize()
    return nc


_NC_CACHE: dict[tuple[int, int], bass.Bass] = {}


def run_spmd(input1: np.ndarray, input2: np.ndarray, **kwargs):
    """Shard, run on 8 cores, gather. Returns (output, BassKernelResults)."""
    x = np.asarray(input1, dtype=np.float32)
    y = np.asarray(input2, dtype=np.float32)
    n, d = x.shape
    m, d2 = y.shape
    assert d == D and d2 == D and n % N_CORES == 0
    rows = n // N_CORES

    # Host-side normalization (matches torch CosineSimilarity eps clamp;
    # norms are ~11 for randn(128), so the clamp never bites here).
    nx = np.maximum(np.sqrt(np.einsum("nd,nd->n", x, x)), 1e-8)
    ny = np.maximum(np.sqrt(np.einsum("nd,nd->n", y, y)), 1e-8)
    xs = (x / nx[:, None]).astype(ml_dtypes.bfloat16)
    ys = (y / ny[:, None]).astype(ml_dtypes.bfloat16)
    xT = np.ascontiguousarray(xs.T)   # [128, n]
    yT = np.ascontiguousarray(ys.T)   # [128, m]

    key = (rows, m)
    if key not in _NC_CACHE:
        _NC_CACHE[key] = build_nc(rows, m)
    nc = _NC_CACHE[key]

    in_maps = [
        {"xT": np.ascontiguousarray(xT[:, c * rows : (c + 1) * rows]), "yT": yT}
        for c in range(N_CORES)
    ]
    res = run_bass_kernel_spmd(nc, in_maps, core_ids=list(range(N_CORES)), **kwargs)
    out = np.concatenate(
        [res.results[c]["out"].astype(np.float32) for c in range(N_CORES)], axis=0
    )
    return out, res


def kernel(input1: np.ndarray, input2: np.ndarray) -> np.ndarray:
    return run_spmd(input1, input2)[0]
